# revision 32
# baseline (speedup 1.0000x reference)
"""AssociativeEmbeddingLoss on 8 TRN2 NeuronCores.

Reference, per image b (C=1, G=128 boxes):
    tl[g] = pred[b, 0, ty[g], tx[g]],  br[g] = target[b, 0, by[g], bx[g]]
    me = (tl + br) / 2
    pull_b = sum((tl-br)^2) / (2N)
    push_b = sum_{i != j} relu(1 - |me_i - me_j|) / (N*(N-1))
    out = (0.25 * sum_b pull_b, 0.25 * sum_b push_b)

Data-parallel over batch, 8 images per core. Only the 2*G*BP scalars the
loss touches are read from the big inputs, via 16 indirect DMAs (one
[128,1] column per image/tensor; the Q7 descriptor-generation cost of
~10ns/element is the hard floor for any SWDGE gather). match is loaded
contiguously; flat indices are computed in the [8(b), 128(g)] layout
(per-partition b*H base via a tensor_scalar operand, f32 exact) and
PE-transposed to the [128(g), 8(b)] layout the gather offsets need.
Per-image compute (me, transpose, ones x me_row broadcast matmul,
|me_j - me_i| on the scalar engine with bias = -me_i, and
relu(1-|d|) = 1 - min(|d|,1) as one DVE min+accumulate) is pipelined
behind the remaining gathers. Final partition sums use a ones-matmul
into PSUM plus an activation whose accumulator emits the scalar. Each
core emits its partial [pull_sum, min_sum]; the host combines the 8
pairs (the unshard step).
"""

import numpy as np

import concourse.bacc as bacc
import concourse.bass as bass
import concourse.mybir as mybir
import concourse.tile as tile
from concourse.bass import IndirectOffsetOnAxis
from concourse.bass_utils import run_bass_kernel_spmd

B, C, H, W = 64, 1, 512, 512
G = 128                 # boxes per image; N = G*C = 128
N = G * C
NCORES = 8
BP = B // NCORES        # images per core
NPIX = BP * H * W
PULL_W, PUSH_W = 0.25, 0.25

F32 = mybir.dt.float32
I32 = mybir.dt.int32
AF = mybir.ActivationFunctionType
ALU = mybir.AluOpType

# consts layout: [:, 0:G] identity; [:, G] ones; [0, G+1 : G+1+G] ones row;
# [0:BP, COL_B8] = b*H
COL_ONES = G
COL_ONESROW = G + 1
COL_B8 = 2 * G + 1
NCONST = 2 * G + 2


def _build_nc():
    nc = bacc.Bacc(
        "TRN2",
        target_bir_lowering=False,
        debug=False,
        enable_asserts=False,
        num_devices=NCORES,
    )
    pred = nc.dram_tensor("pred", [NPIX, 1], F32, kind="ExternalInput")
    targ = nc.dram_tensor("target", [NPIX, 1], F32, kind="ExternalInput")
    match = nc.dram_tensor("match", [BP, G * 4], F32, kind="ExternalInput")
    consts = nc.dram_tensor("consts", [G, NCONST], F32, kind="ExternalInput")
    out = nc.dram_tensor("out", [1, 2], F32, kind="ExternalOutput")

    with tile.TileContext(nc) as tc:
        _kernel_body(nc, tc, pred, targ, match, consts, out)
    nc.compile()
    return nc


def _kernel_body(nc, tc, pred, targ, match, consts, out):
    with (
        tc.tile_pool(name="sb", bufs=1) as sb,
        tc.tile_pool(name="ps", bufs=1, space="PSUM") as ps,
        tc.tile_pool(name="psr", bufs=2, space="PSUM") as psr,
    ):
        # ---- constants + contiguous match load ----
        ct = sb.tile([G, NCONST], F32, tag="ct")
        nc.sync.dma_start(out=ct[:], in_=consts.ap())
        ident = ct[:, 0:G]
        ones = ct[:, COL_ONES : COL_ONES + 1]
        ones_row = ct[0:1, COL_ONESROW : COL_ONESROW + G]
        base8 = ct[0:BP, COL_B8 : COL_B8 + 1]          # [8, 1] value b*H

        t8 = sb.tile([BP, G * 4], F32, tag="t8")
        nc.sync.dma_start(out=t8[:], in_=match.ap())
        t8v = t8[:].rearrange("b (g c) -> b g c", g=G, c=4)

        # ---- flat indices in [8, 128] layout, then PE-transpose to [128, 8]
        def flatidx(name, ysel, xsel):
            f8 = sb.tile([BP, G], F32, tag=name + "_f8")
            nc.vector.tensor_scalar(
                out=f8[:], in0=t8v[:, :, ysel], scalar1=base8, scalar2=float(W),
                op0=ALU.add, op1=ALU.mult,
            )
            nc.vector.tensor_tensor(out=f8[:], in0=f8[:], in1=t8v[:, :, xsel], op=ALU.add)
            fp = psr.tile([G, BP], F32, tag="idxp")
            nc.tensor.transpose(out=fp[:], in_=f8[:], identity=ident[0:BP, 0:BP])
            i = sb.tile([G, BP], I32, tag=name)
            nc.vector.tensor_copy(out=i[:], in_=fp[:])
            return i

        tl_idx = flatidx("tlidx", 0, 1)
        br_idx = flatidx("bridx", 2, 3)

        # ---- gathers + per-image pipeline ----
        tl = sb.tile([G, BP], F32, tag="tl")
        br = sb.tile([G, BP], F32, tag="br")
        me2c = sb.tile([G, BP], F32, tag="me2c")
        negme = sb.tile([G, BP], F32, tag="negme")
        dsub = sb.tile([G, BP], F32, tag="dsub")
        min_cols = sb.tile([G, BP], F32, tag="min_cols")

        for b in range(BP):
            cs = slice(b, b + 1)
            nc.gpsimd.indirect_dma_start(
                out=tl[:, cs], out_offset=None, in_=pred.ap(),
                in_offset=IndirectOffsetOnAxis(ap=tl_idx[:, cs], axis=0),
            )
            nc.gpsimd.indirect_dma_start(
                out=br[:, cs], out_offset=None, in_=targ.ap(),
                in_offset=IndirectOffsetOnAxis(ap=br_idx[:, cs], axis=0),
            )
            # per-image compute, overlapping the remaining gathers
            nc.vector.tensor_sub(dsub[:, cs], tl[:, cs], br[:, cs])
            nc.vector.tensor_add(me2c[:, cs], tl[:, cs], br[:, cs])
            nc.vector.tensor_scalar(
                out=negme[:, cs], in0=me2c[:, cs], scalar1=-0.5, scalar2=None,
                op0=ALU.mult,
            )
            rowp = psr.tile([1, G], F32, tag="rowp")
            nc.tensor.transpose(out=rowp[:], in_=me2c[:, cs], identity=ident)
            merow = sb.tile([1, G], F32, tag=f"merow{b % 2}")
            nc.vector.tensor_scalar(
                out=merow[:], in0=rowp[:], scalar1=0.5, scalar2=None, op0=ALU.mult,
            )
            Rp = psr.tile([G, G], F32, tag="Rp")
            nc.tensor.matmul(
                out=Rp[:], lhsT=ones_row, rhs=merow[:], start=True, stop=True,
            )
            ad = sb.tile([G, G], F32, tag=f"ad{b % 2}")
            nc.scalar.activation(
                out=ad[:], in_=Rp[:], func=AF.Abs, bias=negme[:, cs], scale=1.0,
            )
            nc.vector.tensor_scalar(
                out=ad[:], in0=ad[:], scalar1=1.0, scalar2=0.0,
                op0=ALU.min, op1=ALU.add, accum_out=min_cols[:, cs],
            )

        # ---- tail: per-image sums -> [1, 8] psum rows -> ACT accumulate ----
        sq = sb.tile([G, BP], F32, tag="sq")
        nc.vector.tensor_mul(sq[:], dsub[:], dsub[:])
        fin = ps.tile([1, 2 * BP], F32, tag="fin")
        nc.tensor.matmul(out=fin[0:1, 0:BP], lhsT=ones, rhs=sq[:],
                         start=True, stop=True)
        nc.tensor.matmul(out=fin[0:1, BP : 2 * BP], lhsT=ones, rhs=min_cols[:],
                         start=True, stop=True)
        # pull = c_pull * S; push = (BP*N*(N-1) - minsum) * c_push
        c_pull = PULL_W / (2.0 * N)
        c_push = PUSH_W / (N * (N - 1))
        scr = sb.tile([1, 2 * BP], F32, tag="scr")
        res = sb.tile([1, 2], F32, tag="res")
        nc.scalar.activation(out=scr[0:1, 0:BP], in_=fin[0:1, 0:BP], func=AF.Copy,
                             scale=c_pull, accum_out=res[0:1, 0:1])
        nc.scalar.activation(out=scr[0:1, BP : 2 * BP], in_=fin[0:1, BP : 2 * BP],
                             func=AF.Copy,
                             scale=-c_push,
                             bias=float(BP * N * (N - 1)) * c_push / BP,
                             accum_out=res[0:1, 1:2])
        nc.sync.dma_start(out=out.ap(), in_=res[:])


_NC_CACHE = None


def _get_nc():
    global _NC_CACHE
    if _NC_CACHE is None:
        _NC_CACHE = _build_nc()
    return _NC_CACHE


def _consts():
    c = np.zeros((G, NCONST), dtype=np.float32)
    c[:, 0:G] = np.eye(G, dtype=np.float32)
    c[:, COL_ONES] = 1.0
    c[0, COL_ONESROW : COL_ONESROW + G] = 1.0
    c[0:BP, COL_B8] = np.arange(BP, dtype=np.float32) * H
    return c


def make_in_maps(pred, target, match):
    pred = np.asarray(pred, dtype=np.float32)
    target = np.asarray(target, dtype=np.float32)
    match = np.asarray(match)
    consts = _consts()
    in_maps = []
    for k in range(NCORES):
        sl = slice(k * BP, (k + 1) * BP)
        in_maps.append({
            "pred": np.ascontiguousarray(pred[sl]).reshape(NPIX, 1),
            "target": np.ascontiguousarray(target[sl]).reshape(NPIX, 1),
            "match": np.ascontiguousarray(match[sl]).astype(np.float32).reshape(BP, G * 4),
            "consts": consts,
        })
    return in_maps


def kernel(pred, target, match, _trace=False):
    nc = _get_nc()
    in_maps = make_in_maps(pred, target, match)
    res = run_bass_kernel_spmd(nc, in_maps, core_ids=list(range(NCORES)), trace=_trace)
    total = np.zeros((1, 2), dtype=np.float64)
    for r in res.results:
        total += r["out"].astype(np.float64)
    out = (np.float32(total[0, 0]), np.float32(total[0, 1]))
    if _trace:
        return out, res
    return out


# revision 34
# speedup vs baseline: 1.1582x; 1.1582x over previous
"""AssociativeEmbeddingLoss on 8 TRN2 NeuronCores.

Reference, per image b (C=1, G=128 boxes):
    tl[g] = pred[b, 0, ty[g], tx[g]],  br[g] = target[b, 0, by[g], bx[g]]
    me = (tl + br) / 2
    pull_b = sum((tl-br)^2) / (2N)
    push_b = sum_{i != j} relu(1 - |me_i - me_j|) / (N*(N-1))
    out = (0.25 * sum_b pull_b, 0.25 * sum_b push_b)

Data-parallel over batch, 8 images per core. Only the 2*G*BP scalars the
loss touches are read from the big inputs, via 16 indirect DMAs (one
[128,1] column per image/tensor; the Q7 descriptor-generation cost of
~10ns/element is the hard floor for any SWDGE gather). match is loaded
contiguously; flat indices are computed in the [8(b), 128(g)] layout
(per-partition b*H base via a tensor_scalar operand, f32 exact) and
PE-transposed to the [128(g), 8(b)] layout the gather offsets need.
Per-image compute (me, transpose, ones x me_row broadcast matmul,
|me_j - me_i| on the scalar engine with bias = -me_i, and
relu(1-|d|) = 1 - min(|d|,1) as one DVE min+accumulate) is pipelined
behind the remaining gathers. Final partition sums use a ones-matmul
into PSUM plus an activation whose accumulator emits the scalar. Each
core emits its partial [pull_sum, min_sum]; the host combines the 8
pairs (the unshard step).
"""

import numpy as np

import concourse.bacc as bacc
import concourse.bass as bass
import concourse.mybir as mybir
import concourse.tile as tile
from concourse.bass import IndirectOffsetOnAxis
from concourse.bass_utils import run_bass_kernel_spmd

B, C, H, W = 64, 1, 512, 512
G = 128                 # boxes per image; N = G*C = 128
N = G * C
NCORES = 8
BP = B // NCORES        # images per core
NPIX = BP * H * W
PULL_W, PUSH_W = 0.25, 0.25

F32 = mybir.dt.float32
I32 = mybir.dt.int32
AF = mybir.ActivationFunctionType
ALU = mybir.AluOpType

# consts layout: [:, 0:G] identity; [:, G] ones; [0, G+1 : G+1+G] ones row;
# [0:BP, COL_B8] = b*H
COL_ONES = G
COL_ONESROW = G + 1
COL_B8 = 2 * G + 1
NCONST = 2 * G + 2


def _build_nc():
    nc = bacc.Bacc(
        "TRN2",
        target_bir_lowering=False,
        debug=False,
        enable_asserts=False,
        num_devices=NCORES,
    )
    pred = nc.dram_tensor("pred", [NPIX, 1], F32, kind="ExternalInput")
    targ = nc.dram_tensor("target", [NPIX, 1], F32, kind="ExternalInput")
    match = nc.dram_tensor("match", [BP, G * 4], F32, kind="ExternalInput")
    consts = nc.dram_tensor("consts", [G, NCONST], F32, kind="ExternalInput")
    out = nc.dram_tensor("out", [1, 2], F32, kind="ExternalOutput")

    with tile.TileContext(nc) as tc:
        _kernel_body(nc, tc, pred, targ, match, consts, out)
    nc.compile()
    return nc


def _kernel_body(nc, tc, pred, targ, match, consts, out):
    with (
        tc.tile_pool(name="sb", bufs=1) as sb,
        tc.tile_pool(name="ps", bufs=1, space="PSUM") as ps,
        tc.tile_pool(name="psr", bufs=2, space="PSUM") as psr,
    ):
        # ---- constants + contiguous match load ----
        ct = sb.tile([G, NCONST], F32, tag="ct")
        nc.sync.dma_start(out=ct[:], in_=consts.ap())
        ident = ct[:, 0:G]
        ones = ct[:, COL_ONES : COL_ONES + 1]
        ones_row = ct[0:1, COL_ONESROW : COL_ONESROW + G]
        base8 = ct[0:BP, COL_B8 : COL_B8 + 1]          # [8, 1] value b*H

        t8 = sb.tile([BP, G * 4], F32, tag="t8")
        nc.sync.dma_start(out=t8[:], in_=match.ap())
        t8v = t8[:].rearrange("b (g c) -> b g c", g=G, c=4)

        # ---- flat indices in [8, 128] layout, then PE-transpose to [128, 8]
        def flatidx(name, ysel, xsel):
            f8 = sb.tile([BP, G], F32, tag=name + "_f8")
            nc.vector.tensor_scalar(
                out=f8[:], in0=t8v[:, :, ysel], scalar1=base8, scalar2=float(W),
                op0=ALU.add, op1=ALU.mult,
            )
            nc.vector.tensor_tensor(out=f8[:], in0=f8[:], in1=t8v[:, :, xsel], op=ALU.add)
            fp = psr.tile([G, BP], F32, tag="idxp")
            nc.tensor.transpose(out=fp[:], in_=f8[:], identity=ident[0:BP, 0:BP])
            i = sb.tile([G, BP], I32, tag=name)
            nc.vector.tensor_copy(out=i[:], in_=fp[:])
            return i

        tl_idx = flatidx("tlidx", 0, 1)
        br_idx = flatidx("bridx", 2, 3)

        # ---- gathers + per-image pipeline ----
        tl = sb.tile([G, BP], F32, tag="tl")
        br = sb.tile([G, BP], F32, tag="br")
        me2c = sb.tile([G, BP], F32, tag="me2c")
        negme = sb.tile([G, BP], F32, tag="negme")
        dsub = sb.tile([G, BP], F32, tag="dsub")
        min_cols = sb.tile([G, BP], F32, tag="min_cols")

        for b in range(BP):
            cs = slice(b, b + 1)
            nc.gpsimd.indirect_dma_start(
                out=tl[:, cs], out_offset=None, in_=pred.ap(),
                in_offset=IndirectOffsetOnAxis(ap=tl_idx[:, cs], axis=0),
            )
            nc.gpsimd.indirect_dma_start(
                out=br[:, cs], out_offset=None, in_=targ.ap(),
                in_offset=IndirectOffsetOnAxis(ap=br_idx[:, cs], axis=0),
            )
            # per-image compute, overlapping the remaining gathers
            nc.vector.tensor_sub(dsub[:, cs], tl[:, cs], br[:, cs])
            nc.vector.tensor_add(me2c[:, cs], tl[:, cs], br[:, cs])
            nc.vector.tensor_scalar(
                out=negme[:, cs], in0=me2c[:, cs], scalar1=-0.5, scalar2=None,
                op0=ALU.mult,
            )
            rowp = psr.tile([1, G], F32, tag="rowp")
            nc.tensor.transpose(out=rowp[:], in_=me2c[:, cs], identity=ident)
            merow = sb.tile([1, G], F32, tag=f"merow{b % 2}")
            nc.vector.tensor_scalar(
                out=merow[:], in0=rowp[:], scalar1=0.5, scalar2=None, op0=ALU.mult,
            )
            Rp = psr.tile([G, G], F32, tag="Rp")
            nc.tensor.matmul(
                out=Rp[:], lhsT=ones_row, rhs=merow[:], start=True, stop=True,
            )
            ad = sb.tile([G, G], F32, tag=f"ad{b % 2}")
            nc.scalar.activation(
                out=ad[:], in_=Rp[:], func=AF.Abs, bias=negme[:, cs], scale=1.0,
            )
            nc.vector.tensor_scalar(
                out=ad[:], in0=ad[:], scalar1=1.0, scalar2=0.0,
                op0=ALU.min, op1=ALU.add, accum_out=min_cols[:, cs],
            )

        # ---- tail: per-image sums -> [1, 8] psum rows -> ACT accumulate ----
        sq = sb.tile([G, BP], F32, tag="sq")
        nc.vector.tensor_mul(sq[:], dsub[:], dsub[:])
        fin = ps.tile([1, 2 * BP], F32, tag="fin")
        nc.tensor.matmul(out=fin[0:1, 0:BP], lhsT=ones, rhs=sq[:],
                         start=True, stop=True)
        nc.tensor.matmul(out=fin[0:1, BP : 2 * BP], lhsT=ones, rhs=min_cols[:],
                         start=True, stop=True)
        # pull = c_pull * S; push = (BP*N*(N-1) - minsum) * c_push
        c_pull = PULL_W / (2.0 * N)
        c_push = PUSH_W / (N * (N - 1))
        scr = sb.tile([1, 2 * BP], F32, tag="scr")
        res = sb.tile([1, 2], F32, tag="res")
        nc.scalar.activation(out=scr[0:1, 0:BP], in_=fin[0:1, 0:BP], func=AF.Copy,
                             scale=c_pull, accum_out=res[0:1, 0:1])
        nc.scalar.activation(out=scr[0:1, BP : 2 * BP], in_=fin[0:1, BP : 2 * BP],
                             func=AF.Copy,
                             scale=-c_push,
                             bias=float(BP * N * (N - 1)) * c_push / BP,
                             accum_out=res[0:1, 1:2])
        nc.sync.dma_start(out=out.ap(), in_=res[:])


_NC_CACHE = None


def _get_nc():
    global _NC_CACHE
    if _NC_CACHE is None:
        _NC_CACHE = _build_nc()
    return _NC_CACHE


def _consts():
    c = np.zeros((G, NCONST), dtype=np.float32)
    c[:, 0:G] = np.eye(G, dtype=np.float32)
    c[:, COL_ONES] = 1.0
    c[0, COL_ONESROW : COL_ONESROW + G] = 1.0
    c[0:BP, COL_B8] = np.arange(BP, dtype=np.float32) * H
    return c


def make_in_maps(pred, target, match):
    pred = np.asarray(pred, dtype=np.float32)
    target = np.asarray(target, dtype=np.float32)
    match = np.asarray(match)
    consts = _consts()
    in_maps = []
    for k in range(NCORES):
        sl = slice(k * BP, (k + 1) * BP)
        in_maps.append({
            "pred": np.ascontiguousarray(pred[sl]).reshape(NPIX, 1),
            "target": np.ascontiguousarray(target[sl]).reshape(NPIX, 1),
            "match": np.ascontiguousarray(match[sl]).astype(np.float32).reshape(BP, G * 4),
            "consts": consts,
        })
    return in_maps


def kernel(pred, target, match, _trace=False):
    nc = _get_nc()
    in_maps = make_in_maps(pred, target, match)
    res = run_bass_kernel_spmd(nc, in_maps, core_ids=list(range(NCORES)), trace=_trace)
    total = np.zeros((1, 2), dtype=np.float64)
    for r in res.results:
        total += r["out"].astype(np.float64)
    out = (np.float32(total[0, 0]), np.float32(total[0, 1]))
    if _trace:
        return out, res
    return out
